# revision 15
# baseline (speedup 1.0000x reference)
"""Trainium2 Bass kernel for nn_Attention_82660940579436.

Computation (see reference):
    q     = mean_s(hidden @ Wq.T + bq)            [B, H]
    key   = tanh(hidden @ Wk.T + bk)              [S, B, H]
    score = einsum('bsh,bh->bs', key, q) + mask   [B, S]
    out   = softmax(score) @ key                  [B, H]

Sharding: data-parallel over batch. B=32 over 8 cores -> 4 batches/core.

v7 design (115us v6 -> this).  v6 was PE-bound (89% busy: bf16 matmuls)
with DVE at 78% (stt score ops at 1x + prepass reduction).  v7 moves
nearly everything to cheaper forms:
  - hidden ships as fp8e4 in TWO host-packed layouts:
      xk [chunk, j, (tile, cj, s)]  -- keypass operand, ACTIVE tiles only
      xq [chunk, s, (pair, cj, i, j)] -- transposed, full sequence
  - KEY MATMULS in fp8 DoubleRow perf mode (0.5 cyc/row): 2 matmuls of
    K=256 (107ns each) + a rank-2 DoubleRow bias matmul folding bk
    (107ns) instead of bf16 4x213 + 213 bias.
  - THE PREPASS IS GONE: q's sum over s runs on PE as DoubleRow
    ones-matmuls over the transposed xq layout (free dim 1 -> ~free),
    accumulating 16 (group x jc) PSUM chains.  DVE keeps only scores.
  - tanh batched per PAIR of tiles ([TOK,1024] PSUM -> 1038ns vs 2x612).
  - score mul+rowsum: DVE stt runs at 1x (594ns); tensor_mul (2x) +
    tensor_scalar-accum (4x) = 520ns; a knob-set fraction of tiles runs
    the fused stt on GPSIMD/Pool (806ns) instead, which is otherwise
    idle because ALL DMA goes through HWDGE (sync/SP) in v7 -- SWDGE
    descgen used to cost ~1us of Pool per load.
  - exp batched per quad of tiles ([TOK,4], init-dominated 187ns);
    the -60 mask bias is added by a tiny DVE add first.
  - masked-tile skipping + length-sorted (core, group) assignment and
    per-t_cnts program cache as in v6.

Scheduling: single pass over active tiles (keypass).  xq chunk DMAs and
their qsum matmul blocks are dripped into the keypass at fixed tile
offsets (block ~5 tiles after its DMA so the in-order PE queue never
parks on a DMA wait); each q(g) chain is split into 3 stages a tile
apart so cross-engine waits don't stall the PE/DVE queues.  B work
drains through a global backlog (<=3 tiles per keypass tile, stagger
behind key production), accumulating into one order-free PSUM chain.
exp() needs no max-subtraction: scores are O(1) by construction, masked
positions get -60 bias -> exp underflows to ~1e-26.
"""

import sys
from contextlib import ExitStack

import numpy as np

if "/opt/trn_rl_repo" not in sys.path:
    sys.path.insert(0, "/opt/trn_rl_repo")

import ml_dtypes  # noqa: E402

import concourse.bacc as bacc  # noqa: E402
import concourse.bass as bass  # noqa: E402
import concourse.mybir as mybir  # noqa: E402
import concourse.tile as tile  # noqa: E402
from concourse.bass_utils import run_bass_kernel_spmd  # noqa: E402

S, B, H = 4096, 32, 512
NCORES = 8
BPC = B // NCORES  # 4 batches per core = 4 groups
NT = 128  # tiles per core
TPG = NT // BPC  # 32 tiles per group
TOK = S // TPG  # 128 tokens (s-positions) per tile
HC = H // 128  # 4 chunks of the H (j / i) dims
CHUNK = 8  # tiles per xk DMA
NCHUNK = NT // CHUNK
NPAIR_G = TPG // 2  # 16 qsum pairs per group
XQ_PER_CHUNK = 4  # pairs per xq DMA
NXQ = BPC * NPAIR_G // XQ_PER_CHUNK  # 16 xq chunks (4 per group)
MASK_NEG = -60.0
F32 = mybir.dt.float32
BF16 = mybir.dt.bfloat16
FP8 = mybir.dt.float8e4
AF = mybir.ActivationFunctionType
ALU = mybir.AluOpType
PM = mybir.MatmulPerfMode
BF16NP = ml_dtypes.bfloat16
FP8NP = ml_dtypes.float8_e4m3

# fp8 const pack layout (offsets in elements, [128, PACK8] tensor)
OFF8_ONES = 0  # [128, 2] ones (qsum rhs pair)
OFF8_WA = 2  # [128, 2048] f8(256 WkT) pair chunks
PACK8_URG = 2050  # end of urgent prefix (first tile's WA chain)
OFF8_WC = 2050  # [128, 2048] f8(256 WkT - WA) pair chunks
OFF8_WB = 4098  # [128, 2048] f8(WkT) pair chunks
OFF8_BIASL = 6146  # [1, 256] row0: 1.0 x128 | 1/16 x128 (bias lhsT pair)
OFF8_BKP = 6402  # [1, 1024] row0: f8(16bk) | f8(16(16bk-f8(16bk)))
PACK8 = 7426

# bf16 const pack [128, PACKB]
OFFB_ONESROW = 0  # [1, 128] ones (qrep rank-1 lhsT)
OFFB_ONES1 = 128  # [128, 1] ones (den rhs)
OFFB_OH = 129  # [128, 16] one-hot rows: cols g*4+g' = (g == g')
OFFB_WQ = 160  # [128, 2048] WqT chunks (bf16 q chain)
PACKB = 2208

# fp32 const pack [128, PACKF]
OFF_MASK = 0  # [128, NT] mask bias (0 / MASK_NEG), col=global tile
OFF_BQ = 128  # [1, 512] bq row
OFF_ZERO = 640  # [128, 1] zeros (activation bias)
PACKF = 641

# tuning knobs (read at build time)
KNOBS = {
    "xk_bufs": 4,
    "xq_bufs": 5,
    "keypair_bufs": 32,  # bf16 key tiles ([TOK, 512] each)
    "prod_bufs": 3,
    "prodg_bufs": 2,
    "small_bufs": 8,
    "keyps_bufs": 4,  # PSUM tiles [TOK, 512] (1 bank each)
    "stagger": 5,  # tiles of keypass(g) emitted before B(g) starts
    "drain": 4,  # max backlog tiles drained per keypass tile
    # fraction (in tenths) of tiles whose score op runs on Pool (gpsimd
    # fused stt, 806ns) instead of DVE (TT-mul 327 + TS-accum 193)
    "pool10": 4,
    "xq_t0": 1,  # keypass tile of first xq DMA
    "xq_step": 3,  # keypass tiles between xq DMAs
    "xq_lag": 5,  # tiles between an xq DMA and its qsum matmul block
}


def _build_kernel_body(tc, aps, t_cnts):
    nc = tc.nc
    xk, xq = aps["xk"], aps["xq"]
    p8d, cfd, cbd, y = aps["p8"], aps["cf"], aps["cb"], aps["y"]

    with ExitStack() as ctx:
        consts = ctx.enter_context(tc.tile_pool(name="consts", bufs=1))
        pxk = ctx.enter_context(tc.tile_pool(name="xk", bufs=KNOBS["xk_bufs"]))
        pxq = ctx.enter_context(tc.tile_pool(name="xq", bufs=KNOBS["xq_bufs"]))
        pkeys = ctx.enter_context(
            tc.tile_pool(name="keys", bufs=KNOBS["keypair_bufs"])
        )
        psmall = ctx.enter_context(tc.tile_pool(name="small", bufs=KNOBS["small_bufs"]))
        pprod = ctx.enter_context(tc.tile_pool(name="prod", bufs=KNOBS["prod_bufs"]))
        pprodg = ctx.enter_context(tc.tile_pool(name="prodg", bufs=KNOBS["prodg_bufs"]))
        pacc = ctx.enter_context(tc.tile_pool(name="acc", bufs=1))
        pq = ctx.enter_context(tc.tile_pool(name="q", bufs=1))
        pps_key = ctx.enter_context(
            tc.tile_pool(name="ps_key", bufs=KNOBS["keyps_bufs"], space="PSUM")
        )
        pps_acc = ctx.enter_context(tc.tile_pool(name="ps_acc", bufs=1, space="PSUM"))
        pps_q = ctx.enter_context(tc.tile_pool(name="ps_q", bufs=1, space="PSUM"))
        pps_sm = ctx.enter_context(tc.tile_pool(name="ps_sm", bufs=1, space="PSUM"))

        # ---- constants.  Urgent p8 prefix gates the first key chain; the
        # lazy tail (Wq) plus cf/cb load right after the first xk chunk.
        p8 = consts.tile([128, PACK8], FP8)
        nc.sync.dma_start(p8[:, :PACK8_URG], p8d[:, :PACK8_URG])
        cf = consts.tile([128, PACKF], F32)
        cb = consts.tile([128, PACKB], BF16)

        ones8_pair = p8[:, OFF8_ONES : OFF8_ONES + 2].rearrange(
            "p (i o) -> p i o", i=2, o=1
        )
        biasl = p8[0:1, OFF8_BIASL : OFF8_BIASL + 256].rearrange(
            "p (i s) -> p i s", i=2, s=128
        )
        bkp = p8[0:1, OFF8_BKP : OFF8_BKP + 1024].rearrange(
            "p (i h) -> p i h", i=2, h=512
        )

        def wk8(off, c2):
            return p8[:, off + c2 * 1024 : off + (c2 + 1) * 1024].rearrange(
                "p (i h) -> p i h", i=2, h=512
            )

        def wqb(c):
            return cb[:, OFFB_WQ + c * 512 : OFFB_WQ + (c + 1) * 512]

        maskb_sb = cf[:, OFF_MASK : OFF_MASK + NT]
        bq_sb = cf[0:1, OFF_BQ : OFF_BQ + H]
        zero_sb = cf[:, OFF_ZERO : OFF_ZERO + 1]
        ones_row_sb = cb[0:1, OFFB_ONESROW : OFFB_ONESROW + 128]
        ones1_sb = cb[:, OFFB_ONES1 : OFFB_ONES1 + 1]

        def oh_sb(g):
            return cb[:, OFFB_OH + g * BPC : OFFB_OH + (g + 1) * BPC]

        # Dummy PE op observing the urgent-p8 DMA lane (walrus allows one
        # sync-wait per Matmult; this keeps real matmuls at one wait each).
        scr = pps_sm.tile([128, H], F32, tag="sm")
        nc.tensor.matmul(
            scr[0:1, 0:1], p8[0:1, 0:1], p8[0:1, 0:1], start=True, stop=True
        )

        # qsum: col (g, jc) = sum_s hidden[s, g, jc*128:(jc+1)*128].
        # One contiguous 16-matmul PE chain per column, emitted after BOTH
        # of the group's xq chunks land: interleaved (concurrently open)
        # accumulation chains within one PSUM bank lose updates on hw.
        qsum_ps = pps_q.tile([128, BPC * HC], F32, tag="qsum")

        nb_total = sum(t_cnts)
        state = {"nb": 0}
        keytile = {}  # (g, t) -> sbuf key tile [TOK, 512]
        quads = {}  # (g, t//4) -> sc quad tile [TOK, 4] f32
        qrep = [None] * BPC
        qtmp = {}

        # ---------------- keypass ----------------
        xk_tiles = {}

        def emit_xk_dma(g, t):
            T = g * TPG + t
            ntl = min(CHUNK, t_cnts[g] - t)
            TW = 2 * H  # cols per tile (hidiv | lo8)
            h_t = pxk.tile([128, CHUNK * TW], FP8, tag="xk")
            if T == 0 and ntl > 2:
                nc.sync.dma_start(h_t[:, : 2 * TW], xk[0][:, : 2 * TW])
                nc.sync.dma_start(
                    h_t[:, 2 * TW : ntl * TW], xk[0][:, 2 * TW : ntl * TW]
                )
            else:
                nc.sync.dma_start(h_t[:, : ntl * TW], xk[T // CHUNK][:, : ntl * TW])
            xk_tiles[(g, t // CHUNK)] = h_t

        def emit_key(g, t):
            m = t_cnts[g]
            if t % CHUNK == 0 and (g, t // CHUNK) not in xk_tiles:
                emit_xk_dma(g, t)
            hview = xk_tiles[(g, t // CHUNK)][
                :, (t % CHUNK) * 2 * H : (t % CHUNK + 1) * 2 * H
            ]
            hidiv, lo8 = hview[:, :H], hview[:, H : 2 * H]
            kps = pps_key.tile([TOK, H], F32, tag="key", name=f"kps{g}_{t}")
            # PSUM accumulates 16x the preactivation:
            #   hidiv@WA + hidiv@WC + lo8@WB + (Bhi + Blo/16)
            # WA chain first (start, urgent consts), bias last (stop, lazy)

            def hp(src, c2):
                return src[:, c2 * 256 : (c2 + 1) * 256].rearrange(
                    "p (i s) -> p i s", i=2, s=128
                )

            chains = [(hidiv, OFF8_WA), (hidiv, OFF8_WC), (lo8, OFF8_WB)]
            for ci, (src, off) in enumerate(chains):
                for c2 in range(2):
                    nc.tensor.matmul(
                        kps,
                        hp(src, c2),
                        wk8(off, c2),
                        start=(ci == 0 and c2 == 0),
                        stop=False,
                        perf_mode=PM.DoubleRow,
                    )
            nc.tensor.matmul(kps, biasl, bkp, start=False, stop=True,
                             perf_mode=PM.DoubleRow)
            kp = pkeys.tile([TOK, H], BF16, tag="key")
            nc.scalar.activation(kp, kps, AF.Tanh, bias=zero_sb, scale=1.0 / 16.0)
            keytile[(g, t)] = kp

        def key_view(g, t):
            return keytile[(g, t)]

        # ---------------- B phase ----------------
        def emit_b_score(g, t):
            T = g * TPG + t
            tq = t // 4
            if (g, tq) not in quads:
                quads[(g, tq)] = psmall.tile([TOK, 4], F32, tag="sc", name=f"sc{g}_{tq}")
            sc_col = quads[(g, tq)][:, t % 4 : t % 4 + 1]
            key_t = key_view(g, t)
            if (T * KNOBS["pool10"]) % 10 < KNOBS["pool10"]:
                # Pool can only run TensorTensor: mul there, rowsum on DVE
                prod = pprodg.tile([TOK, H], BF16, tag="prodg")
                nc.gpsimd.tensor_mul(prod, key_t, qrep[g])
            else:
                prod = pprod.tile([TOK, H], BF16, tag="prod")
                nc.vector.tensor_mul(prod, key_t, qrep[g])
            nc.vector.tensor_scalar(
                prod, prod, 1.0, 0.0, op0=ALU.mult, op1=ALU.add,
                accum_out=sc_col,
            )

        def emit_b_finish(g, tq, nq):
            T0 = g * TPG + tq * 4
            sc_q = quads[(g, tq)][:, :nq]
            e_q = psmall.tile([TOK, 4], F32, tag="e", name="e_q")[:, :nq]
            nc.vector.tensor_add(e_q, sc_q, maskb_sb[:, T0 : T0 + nq])
            ee = psmall.tile([TOK, 4], F32, tag="ee", name="ee")[:, :nq]
            nc.scalar.activation(ee, e_q, AF.Exp, bias=zero_sb)
            for i in range(nq):
                ei_t = psmall.tile([TOK, BPC], BF16, tag="ei")
                nc.vector.tensor_scalar_mul(ei_t, oh_sb(g), ee[:, i : i + 1])
                first = state["nb"] == 0
                last = state["nb"] == nb_total - 1
                nc.tensor.matmul(numer_ps, ei_t, key_view(g, tq * 4 + i),
                                 start=first, stop=last)
                nc.tensor.matmul(den_ps, ei_t, ones1_sb, start=first, stop=last)
                state["nb"] += 1

        numer_ps = pps_acc.tile([BPC, H], F32, tag="numer")
        den_ps = pps_acc.tile([BPC, 1], F32, tag="den")

        # ---------------- qsum / q chain ----------------
        xq_tiles = {}

        def emit_xq_dma(c):
            h_t = pxq.tile([128, XQ_PER_CHUNK * 1024], FP8, tag="xq")
            nc.sync.dma_start(h_t, xq[c])
            xq_tiles[c] = h_t

        NCG = NPAIR_G // XQ_PER_CHUNK  # xq chunks per group

        def emit_qsum_group(g):
            for jc in range(HC):
                col = g * HC + jc
                for half in range(NCG):
                    h_t = xq_tiles[NCG * g + half]
                    for p in range(XQ_PER_CHUNK):
                        lhsT = h_t[
                            :, p * 1024 + jc * 256 : p * 1024 + (jc + 1) * 256
                        ].rearrange("p (i j) -> p i j", i=2, j=128)
                        nc.tensor.matmul(
                            qsum_ps[:, col : col + 1],
                            lhsT,
                            ones8_pair,
                            start=(half == 0 and p == 0),
                            stop=(half == NCG - 1 and p == XQ_PER_CHUNK - 1),
                            perf_mode=PM.DoubleRow,
                            skip_group_check=True,
                        )

        def emit_q_a(g):
            # PSUM qsum -> bf16 sum columns (1/S folded into q_b)
            maccb = pq.tile([128, HC], BF16, tag=f"mb{g}")
            nc.vector.tensor_copy(maccb, qsum_ps[:, g * HC : (g + 1) * HC])
            qtmp[("mb", g)] = maccb

        def emit_q_b(g):
            maccb = qtmp[("mb", g)]
            q_ps = pps_sm.tile([128, H], F32, tag="sm")
            for c in range(HC):
                nc.tensor.matmul(
                    q_ps[0:1, :],
                    maccb[:, c : c + 1],
                    wqb(c),
                    start=(c == 0),
                    stop=(c == HC - 1),
                )
            q_b = pq.tile([1, H], BF16, tag=f"qb{g}")
            nc.vector.scalar_tensor_tensor(
                q_b, q_ps[0:1, :], 1.0 / S, bq_sb, ALU.mult, ALU.add,
            )
            qtmp[("qb", g)] = q_b

        def emit_q_c(g):
            q_b = qtmp[("qb", g)]
            qrep_ps = pps_sm.tile([128, H], F32, tag="sm")
            nc.tensor.matmul(qrep_ps, ones_row_sb, q_b, start=True, stop=True)
            qrep_g = pq.tile([128, H], BF16, tag=f"qr{g}")
            nc.vector.tensor_copy(qrep_g, qrep_ps)
            qrep[g] = qrep_g
            q_done[g] = True

        # ---------------- schedule ----------------
        emit_xk_dma(0, 0)
        nc.sync.dma_start(p8[:, PACK8_URG:], p8d[:, PACK8_URG:])
        nc.sync.dma_start(cf, cfd)
        nc.sync.dma_start(cb, cbd)

        def emit_lazy_observers():
            # PE observer for the cb lane (Wq / ones_row / ones1), so the
            # q-chain and B matmuls keep to one sync-wait each.
            nc.tensor.matmul(
                scr[0:1, 0:1], ones1_sb[0:1, :], ones1_sb[0:1, :],
                start=True, stop=True,
            )

        # pending ops keyed by global keypass tile index
        pending = {}

        def at_tile(i, fn):
            pending.setdefault(i, []).append(fn)

        t0, step, lag = KNOBS["xq_t0"], KNOBS["xq_step"], KNOBS["xq_lag"]
        for c in range(NXQ):
            at_tile(t0 + c * step, lambda c=c: emit_xq_dma(c))
            if c % NCG == NCG - 1:
                g = c // NCG
                at_tile(t0 + c * step + lag, lambda g=g: emit_qsum_group(g))
                at_tile(t0 + c * step + lag + 1, lambda g=g: emit_q_a(g))
                at_tile(t0 + c * step + lag + 4, lambda g=g: emit_q_b(g))
                at_tile(t0 + c * step + lag + 7, lambda g=g: emit_q_c(g))
        at_tile(3, emit_lazy_observers)
        # cross-group first-chunk prefetch a few tiles before each window
        wstart = 0
        for g in range(1, BPC):
            wstart += t_cnts[g - 1]
            at_tile(max(1, wstart - 4),
                    lambda g=g: None if (g, 0) in xk_tiles else emit_xk_dma(g, 0))

        q_done = [False] * BPC
        # backlog of (g, t) score ops; quad finishes fire when their last
        # tile's score has been emitted
        backlog = []
        sc_done = {}  # (g, tq) -> count

        def quad_len(g, tq):
            return min(4, t_cnts[g] - tq * 4)

        def drain_one():
            bg, bt = backlog.pop(0)
            emit_b_score(bg, bt)
            tq = bt // 4
            sc_done[(bg, tq)] = sc_done.get((bg, tq), 0) + 1
            if sc_done[(bg, tq)] == quad_len(bg, tq):
                emit_b_finish(bg, tq, quad_len(bg, tq))

        gt = 0
        stg = KNOBS["stagger"]
        for g in range(BPC):
            m = t_cnts[g]
            stg_g = min(stg, max(2, m // 2))
            for t in range(m):
                emit_key(g, t)
                # intra-group prefetch: next chunk at tile 0, next-next at 4
                if t % CHUNK == 0 and t + CHUNK < m:
                    if (g, t // CHUNK + 1) not in xk_tiles:
                        emit_xk_dma(g, (t // CHUNK + 1) * CHUNK)
                if t % CHUNK == 4 and (t // CHUNK + 2) * CHUNK < m:
                    if (g, t // CHUNK + 2) not in xk_tiles:
                        emit_xk_dma(g, (t // CHUNK + 2) * CHUNK)
                for fn in pending.pop(gt, []):
                    fn()
                gt += 1
                nb = 0
                while backlog and nb < KNOBS["drain"]:
                    bg, bt = backlog[0]
                    if bg == g and bt > t - stg_g:
                        break
                    if not q_done[bg]:
                        break
                    drain_one()
                    nb += 1
                backlog.append((g, t))
        # flush remaining pending ops (short groups may end early)
        for i in sorted(pending):
            for fn in pending[i]:
                fn()
        pending.clear()
        while backlog:
            drain_one()

        # ---- out = numer / den ----
        rcp = pacc.tile([BPC, 1], F32)
        nc.vector.reciprocal(rcp, den_ps)
        out_sb = pacc.tile([BPC, H], F32)
        nc.vector.tensor_scalar_mul(out_sb, numer_ps, rcp)
        nc.sync.dma_start(y, out_sb)


_CACHE = {}


def _get_program(t_cnts=None):
    if t_cnts is None:
        t_cnts = _CACHE.get("last")
        assert t_cnts is not None, "no program built yet"
    t_cnts = tuple(int(t) for t in t_cnts)
    if t_cnts in _CACHE:
        _CACHE["last"] = t_cnts
        return _CACHE[t_cnts]
    nc = bacc.Bacc(None, target_bir_lowering=False, debug=False)
    aps = {
        "xk": nc.dram_tensor(
            "xk", [NCHUNK, 128, CHUNK * 2 * H], FP8, kind="ExternalInput"
        ).ap(),
        "xq": nc.dram_tensor(
            "xq", [NXQ, 128, XQ_PER_CHUNK * 1024], FP8, kind="ExternalInput"
        ).ap(),
        "p8": nc.dram_tensor("p8", [128, PACK8], FP8, kind="ExternalInput").ap(),
        "cf": nc.dram_tensor("cf", [128, PACKF], F32, kind="ExternalInput").ap(),
        "cb": nc.dram_tensor("cb", [128, PACKB], BF16, kind="ExternalInput").ap(),
        "y": nc.dram_tensor("y", [BPC, H], F32, kind="ExternalOutput").ap(),
    }
    with tile.TileContext(nc) as tc:
        _build_kernel_body(tc, aps, t_cnts)
    nc.finalize()
    _CACHE[t_cnts] = (nc, aps)
    _CACHE["last"] = t_cnts
    return nc, aps


def _plan(lengths):
    """Sort batches by length (desc); rank r -> (core r%8, group r//8).
    Returns (order, t_cnts)."""
    lens = np.asarray(lengths).astype(np.int64)
    order = np.argsort(-lens, kind="stable")
    sl = lens[order].reshape(BPC, NCORES)  # [group, core]
    t_cnts = np.ceil(sl.max(axis=1) / TOK).astype(int)
    return order, tuple(int(t) for t in t_cnts)


def _make_in_maps(hidden_states, Wq, bq, Wk, bk, lengths, order):
    hidden = np.asarray(hidden_states, dtype=np.float32)
    Wq = np.asarray(Wq, dtype=np.float32)
    Wk = np.asarray(Wk, dtype=np.float32)
    bqv = np.asarray(bq, dtype=np.float32)
    bkv = np.asarray(bk, dtype=np.float32)
    lens = np.asarray(lengths).astype(np.int64)

    p = np.arange(128)
    p8 = np.zeros((128, PACK8), dtype=FP8NP)
    p8[:, OFF8_ONES : OFF8_ONES + 2] = FP8NP(1.0)
    p8[0, OFF8_BIASL : OFF8_BIASL + 128] = FP8NP(1.0)
    p8[0, OFF8_BIASL + 128 : OFF8_BIASL + 256] = FP8NP(1.0 / 16.0)
    bhi = (bkv * 16.0).astype(FP8NP)
    blo = (16.0 * (bkv * 16.0 - bhi.astype(np.float32))).astype(FP8NP)
    p8[0, OFF8_BKP : OFF8_BKP + 512] = bhi
    p8[0, OFF8_BKP + 512 : OFF8_BKP + 1024] = blo

    def wk_pack(w):  # [h, j] fp32 -> [128, 2048] (j, c2, i, h) pair layout
        return (
            np.ascontiguousarray(w.T)  # [j_full, h]
            .reshape(2, 2, 128, H)  # c2, i, j, h
            .transpose(2, 0, 1, 3)  # j, c2, i, h
            .reshape(128, 2048)
        )

    WA = (256.0 * Wk).astype(FP8NP)
    WC = (256.0 * Wk - WA.astype(np.float32)).astype(FP8NP)
    WB = Wk.astype(FP8NP)
    p8[:, OFF8_WA : OFF8_WA + 2048] = wk_pack(WA.astype(np.float32)).astype(FP8NP)
    p8[:, OFF8_WC : OFF8_WC + 2048] = wk_pack(WC.astype(np.float32)).astype(FP8NP)
    p8[:, OFF8_WB : OFF8_WB + 2048] = wk_pack(WB.astype(np.float32)).astype(FP8NP)
    cb = np.zeros((128, PACKB), dtype=BF16NP)
    cb[:, OFFB_WQ : OFFB_WQ + 2048] = (
        np.ascontiguousarray(Wq.T)
        .reshape(HC, 128, H)
        .transpose(1, 0, 2)
        .reshape(128, 2048)
        .astype(BF16NP)
    )
    cb[0, OFFB_ONESROW : OFFB_ONESROW + 128] = BF16NP(1.0)
    cb[:, OFFB_ONES1] = BF16NP(1.0)
    for g in range(BPC):
        cb[:, OFFB_OH + g * BPC + g] = BF16NP(1.0)

    base_cf = np.zeros((128, PACKF), dtype=np.float32)
    base_cf[0, OFF_BQ : OFF_BQ + H] = bqv

    in_maps = []
    t_idx = np.arange(NT)
    for c in range(NCORES):
        bsel = [int(order[g * NCORES + c]) for g in range(BPC)]
        hc = hidden[:, bsel, :]  # [S, 4, 512]
        # xk: [chunk, j, (tile_in_chunk, hilo, cj, s)] fp8 hidiv/lo8 split
        hidiv = hc.astype(np.float32) / 16.0
        hidiv8 = hidiv.astype(FP8NP)
        lo8 = (16.0 * (hc - 16.0 * hidiv8.astype(np.float32))).astype(FP8NP)

        def xk_pack(arr):  # [S, 4, H] -> [NCHUNK, 128, CHUNK, 512]
            return np.ascontiguousarray(
                arr.transpose(1, 0, 2)  # [g, S, H]
                .reshape(BPC, NCHUNK // BPC, CHUNK, TOK, HC, 128)
                .transpose(0, 1, 5, 2, 4, 3)  # g, ch, j, tl, cj, s
            ).reshape(NCHUNK, 128, CHUNK, H)

        xkc = np.concatenate(
            [xk_pack(hidiv8.astype(np.float32)), xk_pack(lo8.astype(np.float32))],
            axis=3,
        ).reshape(NCHUNK, 128, CHUNK * 2 * H).astype(FP8NP)
        # xq: [chunk, s, (pair_in_chunk, cj, i, j)] fp8
        xqc = np.ascontiguousarray(
            hc.transpose(1, 0, 2)  # [g, S, H]
            .reshape(BPC, NPAIR_G, 2, TOK, HC, 128)  # g, pair, i, s, cj, j
            .transpose(0, 1, 3, 4, 2, 5)  # g, pair, s, cj, i, j
            .reshape(NXQ, XQ_PER_CHUNK, 128, 1024)  # chunk, pair, s, cols
            .transpose(0, 2, 1, 3)  # chunk, s, pair, cols
        ).reshape(NXQ, 128, XQ_PER_CHUNK * 1024).astype(FP8NP)
        cfc = base_cf.copy()
        b_of_t = np.array([bsel[g] for g in t_idx // TPG])  # [NT]
        s_full = (t_idx % TPG)[None, :] * TOK + p[:, None]  # [128, NT]
        valid = s_full < lens[b_of_t][None, :]
        cfc[:, OFF_MASK : OFF_MASK + NT] = np.where(valid, 0.0, MASK_NEG)
        in_maps.append({"xk": xkc, "xq": xqc, "p8": p8, "cf": cfc, "cb": cb})
    return in_maps


def run(hidden_states, Wq, bq, Wk, bk, lengths, trace=False):
    """Run on 8 cores; returns (output [B, H] fp32, BassKernelResults)."""
    order, t_cnts = _plan(lengths)
    nc, _ = _get_program(t_cnts)
    in_maps = _make_in_maps(hidden_states, Wq, bq, Wk, bk, lengths, order)
    res = run_bass_kernel_spmd(
        nc, in_maps, core_ids=list(range(NCORES)), trace=trace
    )
    rows = np.concatenate([np.asarray(r["y"]) for r in res.results], axis=0)
    out = np.empty((B, H), dtype=np.float32)
    for c in range(NCORES):
        for g in range(BPC):
            out[int(order[g * NCORES + c])] = rows[c * BPC + g]
    return out, res


def kernel(hidden_states, Wq, bq, Wk, bk, lengths):
    out, _ = run(hidden_states, Wq, bq, Wk, bk, lengths)
    return out
